# revision 3
# baseline (speedup 1.0000x reference)
"""DimeNet edge_init (DimePredictor) Bass/Trainium2 kernel, v2.

Strategy (8 NeuronCores):
  - Triplets sharded to cores by src-row range (125k rows each), then sorted
    per core by (src-window, dst-window) where windows are 31250 rows (int16
    indexable). 4 src-windows x 32 dst-windows = 128 bins/core, each padded
    to a fixed C_BIN = 4224 slots (zero-index padding; max observed bin
    ~4140 for 4M uniform triplets, asserted at prep time).
  - Per bin, TWO InstDMAGatherAnt ops (single_packet=False, elem < 256B is
    fine in non-transpose mode; HBM row stride must be a multiple of 256B):
      src: 96B rows ([42 x fp16 rbf | 3 x f32 o] from a per-core fp16 table
           slice [125000, 128]) gathered by src%31250,
      dst: 16B rows (f32 [x,y,z,0] from a replicated o table [1M, 64])
           gathered by dst%31250.
    Row i of an instruction lands at partition i%128, chunk i//128.
  - Compute per group of 2 bins (66 chunks of 128 lanes): rbf fp16 -> f32 on
    the ACT engine, angle math in f32 on DVE (dot, norms, rsqrt + one Newton
    step), scaled Legendre recurrence via scalar_tensor_tensor, output fp16.
  - Host: layout-only prep (dtype packing, sort/binning permutation of the
    index data) and inverse permutation + exact fp16->f32 widening of the
    output.
"""
import numpy as np

NUM_SPHERICAL = 7
NUM_RADIAL = 6
D_OUT = NUM_SPHERICAL * NUM_RADIAL  # 42
E_ROWS = 1_000_000
T_FULL = 4_000_000
N_CORES = 8
EC = E_ROWS // N_CORES       # 125_000 src rows per core
W = 31_250                   # window rows (int16-indexable)
NSW = EC // W                # 4 src windows per core
NDW = E_ROWS // W            # 32 dst windows
NBIN = NSW * NDW             # 128 bins per core
C_BIN = 4224                 # slots per bin (33 chunks of 128)
CH_BIN = C_BIN // 128        # 33
GB = 2                       # bins per compute group
N_GROUP = NBIN // GB         # 64
K_G = GB * CH_BIN            # 66 chunk-lanes per group
IDXC_BIN = 2 * (C_BIN // 16)          # idx cols per bin (src 264 + dst 264)
IDXC_G = GB * IDXC_BIN                # 1056 per group
OUT_COLS = NBIN * CH_BIN * D_OUT      # 177_408 fp16 cols per core

_CACHE = {}


def _dma_gather_raw(gp, out_ap, in_ap, idxs_ap, num_idxs, elem_size, elem_step,
                    queue_num=0):
    """InstDMAGatherAnt without bass's over-strict elem%256 assert.

    Non-transpose HBM-source gather: elem_size (in elements) arbitrary, the
    table row stride (elem_step * dtype_size) must be a 256B multiple.
    single_packet=False (coalesced single-packet mode caps num_idxs ~1024).
    """
    import concourse.mybir as mybir
    from concourse.bass import exact_div

    assert idxs_ap.dtype == mybir.dt.int16
    assert in_ap.dtype == out_ap.dtype
    stride_bytes = elem_step * mybir.dt.size(in_ap.dtype)
    stride_bytes_256 = exact_div(stride_bytes, 256)
    assert 0 < stride_bytes_256 < 256
    _in_ap = gp.lower_ap_dma(in_ap, for_custom_bir_dma=True)
    assert len(_in_ap) == 1
    _idxs_ap = gp.lower_ap(idxs_ap)
    _out_ap = gp.lower_ap(out_ap)
    return gp.add_instruction(
        mybir.InstDMAGatherAnt(
            name=gp.bass.get_next_instruction_name(),
            ins=[*_in_ap, _idxs_ap, gp.lower_val_access(gp.to_reg(num_idxs))],
            outs=[_out_ap],
            transpose=False,
            num_idxs=num_idxs,
            elem_size=elem_size,
            stride_bytes_256=stride_bytes_256,
            gen_mode=0,
            single_packet=False,
            queue_num=queue_num,
            sbuf_tokens_per_rank=0,
            sbuf_free_dim_per_rank=0,
            sbuf_free_dim_pad_per_rank=0,
            sbuf_byte_offset=0,
        )
    )


def build_program(repeat=1, n_cores=N_CORES):
    import concourse.bacc as bacc
    import concourse.tile as tile
    import concourse.mybir as mybir

    f32 = mybir.dt.float32
    f16 = mybir.dt.float16
    i16 = mybir.dt.int16
    mul = mybir.AluOpType.mult
    add = mybir.AluOpType.add

    nc = bacc.Bacc("TRN2", target_bir_lowering=False, debug=False,
                   num_devices=n_cores)
    tblh = nc.dram_tensor("tblh", [EC, 128], f16, kind="ExternalInput").ap()
    o4p = nc.dram_tensor("o4p", [E_ROWS, 64], f32, kind="ExternalInput").ap()
    idxs = nc.dram_tensor("idxs", [128, N_GROUP * IDXC_G], i16,
                          kind="ExternalInput").ap()
    out = nc.dram_tensor("out", [128, OUT_COLS], f16, kind="ExternalOutput").ap()

    # Scaled Legendre recurrence (see baseline): G_l = c*G_{l-1} - b2_l*G_{l-2},
    # emitted output per l uses qscale_l = coef_l * g_l.
    g = [1.0, 1.0]
    for l in range(2, NUM_SPHERICAL):
        g.append(((2 * l - 1) / l) * g[-1])
    b2 = {}
    for l in range(2, NUM_SPHERICAL):
        a_l = (2 * l - 1) / l
        b_l = (l - 1) / l
        b2[l] = b_l * g[l - 2] / (a_l * g[l - 1])
    coef = [float(np.sqrt((2 * l + 1) / (4.0 * np.pi)).astype(np.float32))
            for l in range(NUM_SPHERICAL)]
    qscale = [coef[l] * g[l] for l in range(NUM_SPHERICAL)]

    K = K_G
    with tile.TileContext(nc) as tc:
        with tc.tile_pool(name="idxp", bufs=3) as idxp, \
             tc.tile_pool(name="ftp", bufs=3) as ftp, \
             tc.tile_pool(name="odp", bufs=3) as odp, \
             tc.tile_pool(name="rbp", bufs=2) as rbp, \
             tc.tile_pool(name="otp", bufs=3) as otp, \
             tc.tile_pool(name="tmp", bufs=2) as tmp:
            for _rep in range(repeat):
                for grp in range(N_GROUP):
                    it = idxp.tile([128, IDXC_G], i16)
                    nc.sync.dma_start(
                        it[:], idxs[:, grp * IDXC_G:(grp + 1) * IDXC_G])

                    ft = ftp.tile([128, K * 48], f16)
                    od = odp.tile([128, K * 4], f32)
                    ft3 = ft[:].rearrange("p (c e) -> p c e", e=48)
                    od3 = od[:].rearrange("p (c e) -> p c e", e=4)
                    for m in range(GB):
                        b = grp * GB + m
                        iwin, jwin = b // NDW, b % NDW
                        c0 = m * IDXC_BIN
                        _dma_gather_raw(
                            nc.gpsimd,
                            out_ap=ft3[:, m * CH_BIN:(m + 1) * CH_BIN, :],
                            in_ap=tblh[iwin * W:(iwin + 1) * W, 0:48],
                            idxs_ap=it[:, c0:c0 + C_BIN // 16],
                            num_idxs=C_BIN, elem_size=48, elem_step=128)
                        _dma_gather_raw(
                            nc.gpsimd,
                            out_ap=od3[:, m * CH_BIN:(m + 1) * CH_BIN, :],
                            in_ap=o4p[jwin * W:(jwin + 1) * W, 0:4],
                            idxs_ap=it[:, c0 + C_BIN // 16:c0 + IDXC_BIN],
                            num_idxs=C_BIN, elem_size=4, elem_step=64)

                    # rbf fp16 -> f32 on the ACT engine (DVE stays on math)
                    rb = rbp.tile([128, K * D_OUT], f32)
                    rb3 = rb[:].rearrange("p (c e) -> p c e", e=D_OUT)
                    nc.scalar.copy(out=rb[:], in_=ft3[:, :, 0:D_OUT])

                    ff3 = ft[:].bitcast(f32).rearrange("p (c e) -> p c e", e=24)
                    R1 = ff3[:, :, 21:24]
                    R2 = od3[:, :, 0:3]

                    m_t = tmp.tile([128, K * 3], f32, tag="m")
                    m3 = m_t[:].rearrange("p (c e) -> p c e", e=3)
                    sc = tmp.tile([128, K * 10], f32, tag="sc")

                    def lane(i):
                        return sc[:, i * K:(i + 1) * K]

                    dot, n1, n2, p_, r_, t_, cc = (lane(i) for i in range(7))
                    gl = [lane(7), lane(8), lane(9)]  # rotating G lanes

                    def lanes_b(ap_flat):
                        return ap_flat.rearrange(
                            "p (c one) -> p c one", one=1).to_broadcast(
                            [128, K, NUM_RADIAL])

                    # dot = R1.R2 ; n1 = |R1|^2 ; n2 = |R2|^2
                    nc.vector.tensor_tensor(out=m3[:], in0=R1, in1=R2, op=mul)
                    nc.vector.tensor_tensor(out=dot, in0=m_t[:, 0::3],
                                            in1=m_t[:, 1::3], op=add)
                    nc.vector.tensor_tensor(out=dot, in0=dot,
                                            in1=m_t[:, 2::3], op=add)
                    nc.vector.tensor_tensor(out=m3[:], in0=R1, in1=R1, op=mul)
                    nc.vector.tensor_tensor(out=n1, in0=m_t[:, 0::3],
                                            in1=m_t[:, 1::3], op=add)
                    nc.vector.tensor_tensor(out=n1, in0=n1,
                                            in1=m_t[:, 2::3], op=add)
                    nc.vector.tensor_tensor(out=m3[:], in0=R2, in1=R2, op=mul)
                    nc.vector.tensor_tensor(out=n2, in0=m_t[:, 0::3],
                                            in1=m_t[:, 1::3], op=add)
                    nc.vector.tensor_tensor(out=n2, in0=n2,
                                            in1=m_t[:, 2::3], op=add)
                    # cc = dot * rsqrt(n1*n2): ACT sqrt + DVE recip + 1 Newton
                    nc.vector.tensor_tensor(out=p_, in0=n1, in1=n2, op=mul)
                    nc.scalar.sqrt(out=r_, in_=p_)
                    nc.vector.reciprocal(out=r_, in_=r_)
                    nc.vector.tensor_tensor(out=t_, in0=r_, in1=r_, op=mul)
                    nc.vector.tensor_tensor(out=t_, in0=t_, in1=p_, op=mul)
                    nc.vector.tensor_scalar(out=t_, in0=t_, scalar1=-0.5,
                                            scalar2=1.5, op0=mul, op1=add)
                    nc.vector.tensor_tensor(out=r_, in0=r_, in1=t_, op=mul)
                    nc.vector.tensor_tensor(out=cc, in0=dot, in1=r_, op=mul)

                    ot = otp.tile([128, K * D_OUT], f16)
                    ot3 = ot[:].rearrange("p (c e) -> p c e", e=D_OUT)

                    def emit(l, G_ap):
                        # out_l = (rbf_l * qscale_l) * G_l
                        nc.vector.scalar_tensor_tensor(
                            out=ot3[:, :, l * NUM_RADIAL:(l + 1) * NUM_RADIAL],
                            in0=rb3[:, :, l * NUM_RADIAL:(l + 1) * NUM_RADIAL],
                            scalar=float(qscale[l]),
                            in1=lanes_b(G_ap),
                            op0=mul, op1=mul)

                    # l = 0: G_0 = 1
                    nc.vector.tensor_scalar(
                        out=ot3[:, :, 0:NUM_RADIAL],
                        in0=rb3[:, :, 0:NUM_RADIAL],
                        scalar1=float(qscale[0]), scalar2=None, op0=mul)
                    # l = 1: G_1 = cc
                    emit(1, cc)
                    # l = 2: G_2 = cc*cc - b2_2
                    nc.vector.tensor_tensor(out=gl[0], in0=cc, in1=cc, op=mul)
                    nc.vector.tensor_scalar(out=gl[0], in0=gl[0],
                                            scalar1=float(-b2[2]),
                                            scalar2=None, op0=add)
                    emit(2, gl[0])
                    # l = 3: G_3 = cc*G_2 - b2_3*G_1 (G_1 = cc)
                    nc.vector.tensor_tensor(out=t_, in0=cc, in1=gl[0], op=mul)
                    nc.vector.scalar_tensor_tensor(
                        out=gl[1], in0=cc, scalar=float(-b2[3]), in1=t_,
                        op0=mul, op1=add)
                    emit(3, gl[1])
                    # l >= 4: G_l = cc*G_{l-1} - b2_l*G_{l-2}
                    for l in range(4, NUM_SPHERICAL):
                        gm1 = gl[(l - 3) % 3]
                        gm2 = gl[(l - 4) % 3]
                        gcur = gl[(l - 2) % 3]
                        nc.vector.tensor_tensor(out=t_, in0=cc, in1=gm1, op=mul)
                        nc.vector.scalar_tensor_tensor(
                            out=gcur, in0=gm2, scalar=float(-b2[l]), in1=t_,
                            op0=mul, op1=add)
                        emit(l, gcur)

                    nc.sync.dma_start(
                        out[:, grp * K * D_OUT:(grp + 1) * K * D_OUT], ot[:])

    nc.compile()
    return nc


def _get_runner(nc, n_cores):
    """Build a jitted SPMD executor for the compiled Bass program."""
    import jax
    import jax.numpy as jnp
    from jax.sharding import Mesh, PartitionSpec, NamedSharding
    from jax.experimental.shard_map import shard_map
    import concourse.mybir as mybir
    from concourse.bass2jax import _bass_exec_p, install_neuronx_cc_hook, partition_id_tensor

    install_neuronx_cc_hook()
    partition_name = nc.partition_id_tensor.name if nc.partition_id_tensor else None
    in_names, out_names, out_avals = [], [], []
    for alloc in nc.m.functions[0].allocations:
        if not isinstance(alloc, mybir.MemoryLocationSet):
            continue
        name = alloc.memorylocations[0].name
        if alloc.kind == "ExternalInput":
            if name != partition_name:
                in_names.append(name)
        elif alloc.kind == "ExternalOutput":
            out_names.append(name)
            out_avals.append(jax.core.ShapedArray(
                tuple(alloc.tensor_shape), mybir.dt.np(alloc.dtype)))
    n_params = len(in_names)
    n_outs = len(out_avals)
    all_in_names = in_names + out_names
    if partition_name is not None:
        all_in_names = all_in_names + [partition_name]
    donate = tuple(range(n_params, n_params + n_outs))

    def _body(*args):
        operands = list(args)
        if partition_name is not None:
            operands.append(partition_id_tensor())
        outs = _bass_exec_p.bind(
            *operands,
            out_avals=tuple(out_avals),
            in_names=tuple(all_in_names),
            out_names=tuple(out_names),
            lowering_input_output_aliases=(),
            sim_require_finite=True,
            sim_require_nnan=True,
            nc=nc,
        )
        return tuple(outs)

    try:
        devices = jax.devices("axon")[:n_cores]
    except RuntimeError:
        devices = jax.devices()[:n_cores]
    mesh = Mesh(np.asarray(devices), ("core",))
    sharded = jax.jit(
        shard_map(_body, mesh=mesh,
                  in_specs=(PartitionSpec("core"),) * (n_params + n_outs),
                  out_specs=(PartitionSpec("core"),) * n_outs,
                  check_rep=False),
        donate_argnums=donate,
        keep_unused=True,
    )
    shard0 = NamedSharding(mesh, PartitionSpec("core"))

    def make_zeros():
        return [
            jax.jit(
                lambda shape=av.shape, dt=av.dtype: jnp.zeros(
                    (n_cores * shape[0],) + tuple(shape[1:]), dt),
                out_shardings=shard0,
            )()
            for av in out_avals
        ]

    return sharded, in_names, out_names, out_avals, shard0, make_zeros


def _sort_plan(src, dst):
    """Shared by prep_inputs / assemble_output: the (core, bin) sort."""
    core = src // EC
    iwin = (src % EC) // W
    jwin = dst // W
    key = core * NBIN + iwin * NDW + jwin
    order = np.argsort(key, kind="stable")
    key_s = key[order]
    counts = np.bincount(key, minlength=N_CORES * NBIN)
    mx = int(counts.max())
    if mx > C_BIN:
        raise ValueError(
            f"bin overflow: max bin {mx} > C_BIN {C_BIN}; "
            "raise C_BIN (multiple of 128) and rebuild")
    starts = np.zeros(N_CORES * NBIN, dtype=np.int64)
    np.cumsum(counts[:-1], out=starts[1:])
    r = np.arange(T_FULL, dtype=np.int64) - np.repeat(starts, counts)
    return order, key_s, r


def prep_inputs(o, rbf_env, src_idx, dst_idx):
    """Host-side layout-only prep: dtype packing + sort/bin permutation."""
    o = np.asarray(o, dtype=np.float32)
    rbf = np.asarray(rbf_env, dtype=np.float32)
    src = np.asarray(src_idx).astype(np.int64)
    dst = np.asarray(dst_idx).astype(np.int64)
    assert o.shape == (E_ROWS, 3) and rbf.shape == (E_ROWS, D_OUT)
    assert src.shape == (T_FULL,) and dst.shape == (T_FULL,)

    tblh = np.zeros((E_ROWS, 128), dtype=np.float16)
    tblh[:, :D_OUT] = rbf.astype(np.float16)
    tblh[:, D_OUT:48] = o.view(np.float16)  # raw f32 bytes as 6 fp16 lanes
    o4p = np.zeros((E_ROWS, 64), dtype=np.float32)
    o4p[:, :3] = o

    order, key_s, r = _sort_plan(src, dst)
    _CACHE["plan"] = (order, key_s, r)

    s_loc = (src % W).astype(np.int16)[order]
    d_loc = (dst % W).astype(np.int16)[order]
    S = np.zeros(N_CORES * NBIN * C_BIN, dtype=np.int16)
    D = np.zeros(N_CORES * NBIN * C_BIN, dtype=np.int16)
    flat_pos = key_s * C_BIN + r
    S[flat_pos] = s_loc
    D[flat_pos] = d_loc
    # wrap [bins, C_BIN] -> [bins, 16, C_BIN/16] -> replicate to 128 partitions
    Sw = S.reshape(-1, C_BIN // 16, 16).transpose(0, 2, 1)
    Dw = D.reshape(-1, C_BIN // 16, 16).transpose(0, 2, 1)
    # per (core,bin): [2, 16, 264] (src block then dst block)
    SD = np.stack([Sw, Dw], axis=1)  # [8*128bins, 2, 16, 264]
    SD = SD.reshape(N_CORES, NBIN, 2, 16, C_BIN // 16)
    # -> [core, 16, bins, 2, 264] -> [core, 16, NBIN*528]
    SD = SD.transpose(0, 3, 1, 2, 4).reshape(N_CORES, 16, NBIN * IDXC_BIN)
    idxs_feed = np.tile(SD, (1, 8, 1)).reshape(N_CORES * 128, NBIN * IDXC_BIN)

    concat = {
        "tblh": tblh.reshape(N_CORES * EC, 128),
        "o4p": np.concatenate([o4p] * N_CORES, axis=0),
        "idxs": np.ascontiguousarray(idxs_feed),
    }
    return concat


def assemble_output(out_concat):
    """out_concat: [N_CORES*128, OUT_COLS] fp16 -> [T_FULL, 42] f32."""
    order, key_s, r = _CACHE["plan"]
    res = np.asarray(out_concat)
    # rows = [core, part], cols = [bin, chunk, 42]
    R5 = res.reshape(N_CORES, 128, NBIN, CH_BIN, D_OUT)
    dev = np.ascontiguousarray(R5.transpose(0, 2, 3, 1, 4)).reshape(
        N_CORES * NBIN * C_BIN, D_OUT)
    gathered = dev[key_s * C_BIN + r].astype(np.float32)
    out = np.empty((T_FULL, D_OUT), dtype=np.float32)
    out[order] = gathered
    return out


def kernel(o, rbf_env, src_idx, dst_idx):
    import jax

    if "prog" not in _CACHE:
        _CACHE["prog"] = build_program()
        _CACHE["runner"] = _get_runner(_CACHE["prog"], N_CORES)
    sharded, in_names, out_names, out_avals, shard0, make_zeros = _CACHE["runner"]

    concat = prep_inputs(o, rbf_env, src_idx, dst_idx)
    dev_in = [jax.device_put(concat[name], shard0) for name in in_names]
    outs = sharded(*dev_in, *make_zeros())
    jax.block_until_ready(outs)
    out_concat = np.asarray(outs[out_names.index("out")])
    return assemble_output(out_concat)


# revision 7
# speedup vs baseline: 2.5478x; 2.5478x over previous
"""DimeNet edge_init (DimePredictor) Bass/Trainium2 kernel, v2.

Strategy (8 NeuronCores):
  - Triplets sharded to cores by src-row range (125k rows each), then sorted
    per core by (src-window, dst-window) where windows are 31250 rows (int16
    indexable). 4 src-windows x 32 dst-windows = 128 bins/core, each padded
    to a fixed C_BIN = 4224 slots (zero-index padding; max observed bin
    ~4140 for 4M uniform triplets, asserted at prep time).
  - Per bin, TWO InstDMAGatherAnt ops (single_packet=False, elem < 256B is
    fine in non-transpose mode; HBM row stride must be a multiple of 256B):
      src: 96B rows ([42 x fp16 rbf | 3 x f32 o] from a per-core fp16 table
           slice [125000, 128]) gathered by src%31250,
      dst: 16B rows (f32 [x,y,z,0] from a replicated o table [1M, 64])
           gathered by dst%31250.
    Row i of an instruction lands at partition i%128, chunk i//128.
  - Compute per group of 2 bins (66 chunks of 128 lanes): rbf fp16 -> f32 on
    the ACT engine, angle math in f32 on DVE (dot, norms, rsqrt + one Newton
    step), scaled Legendre recurrence via scalar_tensor_tensor, output fp16.
  - Host: layout-only prep (dtype packing, sort/binning permutation of the
    index data) and inverse permutation + exact fp16->f32 widening of the
    output.
"""
import numpy as np

NUM_SPHERICAL = 7
NUM_RADIAL = 6
D_OUT = NUM_SPHERICAL * NUM_RADIAL  # 42
E_ROWS = 1_000_000
T_FULL = 4_000_000
N_CORES = 8
EC = E_ROWS // N_CORES       # 125_000 src rows per core
W = 31_250                   # window rows (int16-indexable)
NSW = EC // W                # 4 src windows per core
NDW = E_ROWS // W            # 32 dst windows
NBIN = NSW * NDW             # 128 bins per core
C_BIN = 4224                 # slots per bin (33 chunks of 128)
CH_BIN = C_BIN // 128        # 33
GB = 2                       # bins per compute group
N_GROUP = NBIN // GB         # 64
K_G = GB * CH_BIN            # 66 chunk-lanes per group
IDXC_BIN = 2 * (C_BIN // 16)          # idx cols per bin (src 264 + dst 264)
IDXC_G = GB * IDXC_BIN                # 1056 per group
OUT_COLS = NBIN * CH_BIN * D_OUT      # 177_408 fp16 cols per core

_CACHE = {}


def _dma_gather_raw(gp, out_ap, in_ap, idxs_ap, num_idxs, elem_size, elem_step,
                    queue_num=0, single_packet=True):
    """InstDMAGatherAnt without bass's over-strict elem%256 assert.

    Non-transpose HBM-source gather: elem_size (in elements) arbitrary, the
    table row stride (elem_step * dtype_size) must be a 256B multiple.
    single_packet=True coalesces each engine's descriptors into one SDMA
    packet (64-desc packet cap => num_idxs <= 1024); False makes each
    descriptor its own packet (~140ns/packet on HW - slow for small elems).
    """
    import concourse.mybir as mybir
    from concourse.bass import exact_div

    assert idxs_ap.dtype == mybir.dt.int16
    assert in_ap.dtype == out_ap.dtype
    stride_bytes = elem_step * mybir.dt.size(in_ap.dtype)
    stride_bytes_256 = exact_div(stride_bytes, 256)
    assert 0 < stride_bytes_256 < 256
    _in_ap = gp.lower_ap_dma(in_ap, for_custom_bir_dma=True)
    assert len(_in_ap) == 1
    _idxs_ap = gp.lower_ap(idxs_ap)
    _out_ap = gp.lower_ap(out_ap)
    return gp.add_instruction(
        mybir.InstDMAGatherAnt(
            name=gp.bass.get_next_instruction_name(),
            ins=[*_in_ap, _idxs_ap, gp.lower_val_access(gp.to_reg(num_idxs))],
            outs=[_out_ap],
            transpose=False,
            num_idxs=num_idxs,
            elem_size=elem_size,
            stride_bytes_256=stride_bytes_256,
            gen_mode=0,
            single_packet=single_packet,
            queue_num=queue_num,
            sbuf_tokens_per_rank=0,
            sbuf_free_dim_per_rank=0,
            sbuf_free_dim_pad_per_rank=0,
            sbuf_byte_offset=0,
        )
    )


def build_program(repeat=1, n_cores=N_CORES, c_instr=1024, n_queues=4):
    import concourse.bacc as bacc
    import concourse.tile as tile
    import concourse.mybir as mybir

    f32 = mybir.dt.float32
    f16 = mybir.dt.float16
    i16 = mybir.dt.int16
    mul = mybir.AluOpType.mult
    add = mybir.AluOpType.add

    # per-bin instruction split: c_instr-sized pieces of the C_BIN slot list
    splits = []
    off = 0
    while off < C_BIN:
        sz = min(c_instr, C_BIN - off)
        assert sz % 128 == 0
        splits.append((off, sz))
        off += sz

    nc = bacc.Bacc("TRN2", target_bir_lowering=False, debug=False,
                   num_devices=n_cores, num_swdge_queues=n_queues)
    tblh = nc.dram_tensor("tblh", [EC, 128], f16, kind="ExternalInput").ap()
    o4p = nc.dram_tensor("o4p", [E_ROWS, 64], f32, kind="ExternalInput").ap()
    idxs = nc.dram_tensor("idxs", [128, N_GROUP * IDXC_G], i16,
                          kind="ExternalInput").ap()
    out = nc.dram_tensor("out", [128, OUT_COLS], f16, kind="ExternalOutput").ap()

    # Scaled Legendre recurrence (see baseline): G_l = c*G_{l-1} - b2_l*G_{l-2},
    # emitted output per l uses qscale_l = coef_l * g_l.
    g = [1.0, 1.0]
    for l in range(2, NUM_SPHERICAL):
        g.append(((2 * l - 1) / l) * g[-1])
    b2 = {}
    for l in range(2, NUM_SPHERICAL):
        a_l = (2 * l - 1) / l
        b_l = (l - 1) / l
        b2[l] = b_l * g[l - 2] / (a_l * g[l - 1])
    coef = [float(np.sqrt((2 * l + 1) / (4.0 * np.pi)).astype(np.float32))
            for l in range(NUM_SPHERICAL)]
    qscale = [coef[l] * g[l] for l in range(NUM_SPHERICAL)]

    K = K_G
    rrq = [0]  # round-robin SWDGE queue assignment for gather instructions
    with tile.TileContext(nc) as tc:
        with tc.tile_pool(name="idxp", bufs=3) as idxp, \
             tc.tile_pool(name="ftp", bufs=3) as ftp, \
             tc.tile_pool(name="odp", bufs=3) as odp, \
             tc.tile_pool(name="rbp", bufs=2) as rbp, \
             tc.tile_pool(name="otp", bufs=3) as otp, \
             tc.tile_pool(name="tmp", bufs=2) as tmp:
            for _rep in range(repeat):
                for grp in range(N_GROUP):
                    it = idxp.tile([128, IDXC_G], i16)
                    nc.sync.dma_start(
                        it[:], idxs[:, grp * IDXC_G:(grp + 1) * IDXC_G])

                    ft = ftp.tile([128, K * 48], f16)
                    od = odp.tile([128, K * 4], f32)
                    ft3 = ft[:].rearrange("p (c e) -> p c e", e=48)
                    od3 = od[:].rearrange("p (c e) -> p c e", e=4)
                    for m in range(GB):
                        b = grp * GB + m
                        iwin, jwin = b // NDW, b % NDW
                        c0 = m * IDXC_BIN
                        for (soff, ssz) in splits:
                            ch0 = m * CH_BIN + soff // 128
                            nch = ssz // 128
                            _dma_gather_raw(
                                nc.gpsimd,
                                out_ap=ft3[:, ch0:ch0 + nch, :],
                                in_ap=tblh[iwin * W:(iwin + 1) * W, 0:48],
                                idxs_ap=it[:, c0 + soff // 16:
                                           c0 + soff // 16 + ssz // 16],
                                num_idxs=ssz, elem_size=48, elem_step=128,
                                queue_num=rrq[0])
                            rrq[0] = (rrq[0] + 1) % n_queues
                            cd0 = c0 + C_BIN // 16 + soff // 16
                            _dma_gather_raw(
                                nc.gpsimd,
                                out_ap=od3[:, ch0:ch0 + nch, :],
                                in_ap=o4p[jwin * W:(jwin + 1) * W, 0:4],
                                idxs_ap=it[:, cd0:cd0 + ssz // 16],
                                num_idxs=ssz, elem_size=4, elem_step=64,
                                queue_num=rrq[0])
                            rrq[0] = (rrq[0] + 1) % n_queues

                    # rbf fp16 -> f32 on the ACT engine (DVE stays on math)
                    rb = rbp.tile([128, K * D_OUT], f32)
                    rb3 = rb[:].rearrange("p (c e) -> p c e", e=D_OUT)
                    nc.scalar.copy(out=rb[:], in_=ft3[:, :, 0:D_OUT])

                    ff3 = ft[:].bitcast(f32).rearrange("p (c e) -> p c e", e=24)
                    R1 = ff3[:, :, 21:24]
                    R2 = od3[:, :, 0:3]

                    m_t = tmp.tile([128, K * 3], f32, tag="m")
                    m3 = m_t[:].rearrange("p (c e) -> p c e", e=3)
                    sc = tmp.tile([128, K * 10], f32, tag="sc")

                    def lane(i):
                        return sc[:, i * K:(i + 1) * K]

                    dot, n1, n2, p_, r_, t_, cc = (lane(i) for i in range(7))
                    gl = [lane(7), lane(8), lane(9)]  # rotating G lanes

                    def lanes_b(ap_flat):
                        return ap_flat.rearrange(
                            "p (c one) -> p c one", one=1).to_broadcast(
                            [128, K, NUM_RADIAL])

                    # dot = R1.R2 ; n1 = |R1|^2 ; n2 = |R2|^2
                    nc.vector.tensor_tensor(out=m3[:], in0=R1, in1=R2, op=mul)
                    nc.vector.tensor_tensor(out=dot, in0=m_t[:, 0::3],
                                            in1=m_t[:, 1::3], op=add)
                    nc.vector.tensor_tensor(out=dot, in0=dot,
                                            in1=m_t[:, 2::3], op=add)
                    nc.vector.tensor_tensor(out=m3[:], in0=R1, in1=R1, op=mul)
                    nc.vector.tensor_tensor(out=n1, in0=m_t[:, 0::3],
                                            in1=m_t[:, 1::3], op=add)
                    nc.vector.tensor_tensor(out=n1, in0=n1,
                                            in1=m_t[:, 2::3], op=add)
                    nc.vector.tensor_tensor(out=m3[:], in0=R2, in1=R2, op=mul)
                    nc.vector.tensor_tensor(out=n2, in0=m_t[:, 0::3],
                                            in1=m_t[:, 1::3], op=add)
                    nc.vector.tensor_tensor(out=n2, in0=n2,
                                            in1=m_t[:, 2::3], op=add)
                    # cc = dot * rsqrt(n1*n2): ACT sqrt + DVE recip + 1 Newton
                    nc.vector.tensor_tensor(out=p_, in0=n1, in1=n2, op=mul)
                    nc.scalar.sqrt(out=r_, in_=p_)
                    nc.vector.reciprocal(out=r_, in_=r_)
                    nc.vector.tensor_tensor(out=t_, in0=r_, in1=r_, op=mul)
                    nc.vector.tensor_tensor(out=t_, in0=t_, in1=p_, op=mul)
                    nc.vector.tensor_scalar(out=t_, in0=t_, scalar1=-0.5,
                                            scalar2=1.5, op0=mul, op1=add)
                    nc.vector.tensor_tensor(out=r_, in0=r_, in1=t_, op=mul)
                    nc.vector.tensor_tensor(out=cc, in0=dot, in1=r_, op=mul)

                    ot = otp.tile([128, K * D_OUT], f16)
                    ot3 = ot[:].rearrange("p (c e) -> p c e", e=D_OUT)

                    def emit(l, G_ap):
                        # out_l = (rbf_l * qscale_l) * G_l
                        nc.vector.scalar_tensor_tensor(
                            out=ot3[:, :, l * NUM_RADIAL:(l + 1) * NUM_RADIAL],
                            in0=rb3[:, :, l * NUM_RADIAL:(l + 1) * NUM_RADIAL],
                            scalar=float(qscale[l]),
                            in1=lanes_b(G_ap),
                            op0=mul, op1=mul)

                    # l = 0: G_0 = 1
                    nc.vector.tensor_scalar(
                        out=ot3[:, :, 0:NUM_RADIAL],
                        in0=rb3[:, :, 0:NUM_RADIAL],
                        scalar1=float(qscale[0]), scalar2=None, op0=mul)
                    # l = 1: G_1 = cc
                    emit(1, cc)
                    # l = 2: G_2 = cc*cc - b2_2
                    nc.vector.tensor_tensor(out=gl[0], in0=cc, in1=cc, op=mul)
                    nc.vector.tensor_scalar(out=gl[0], in0=gl[0],
                                            scalar1=float(-b2[2]),
                                            scalar2=None, op0=add)
                    emit(2, gl[0])
                    # l = 3: G_3 = cc*G_2 - b2_3*G_1 (G_1 = cc)
                    nc.vector.tensor_tensor(out=t_, in0=cc, in1=gl[0], op=mul)
                    nc.vector.scalar_tensor_tensor(
                        out=gl[1], in0=cc, scalar=float(-b2[3]), in1=t_,
                        op0=mul, op1=add)
                    emit(3, gl[1])
                    # l >= 4: G_l = cc*G_{l-1} - b2_l*G_{l-2}
                    for l in range(4, NUM_SPHERICAL):
                        gm1 = gl[(l - 3) % 3]
                        gm2 = gl[(l - 4) % 3]
                        gcur = gl[(l - 2) % 3]
                        nc.vector.tensor_tensor(out=t_, in0=cc, in1=gm1, op=mul)
                        nc.vector.scalar_tensor_tensor(
                            out=gcur, in0=gm2, scalar=float(-b2[l]), in1=t_,
                            op0=mul, op1=add)
                        emit(l, gcur)

                    nc.sync.dma_start(
                        out[:, grp * K * D_OUT:(grp + 1) * K * D_OUT], ot[:])

    nc.compile()
    return nc


def _get_runner(nc, n_cores):
    """Build a jitted SPMD executor for the compiled Bass program."""
    import jax
    import jax.numpy as jnp
    from jax.sharding import Mesh, PartitionSpec, NamedSharding
    from jax.experimental.shard_map import shard_map
    import concourse.mybir as mybir
    from concourse.bass2jax import _bass_exec_p, install_neuronx_cc_hook, partition_id_tensor

    install_neuronx_cc_hook()
    partition_name = nc.partition_id_tensor.name if nc.partition_id_tensor else None
    in_names, out_names, out_avals = [], [], []
    for alloc in nc.m.functions[0].allocations:
        if not isinstance(alloc, mybir.MemoryLocationSet):
            continue
        name = alloc.memorylocations[0].name
        if alloc.kind == "ExternalInput":
            if name != partition_name:
                in_names.append(name)
        elif alloc.kind == "ExternalOutput":
            out_names.append(name)
            out_avals.append(jax.core.ShapedArray(
                tuple(alloc.tensor_shape), mybir.dt.np(alloc.dtype)))
    n_params = len(in_names)
    n_outs = len(out_avals)
    all_in_names = in_names + out_names
    if partition_name is not None:
        all_in_names = all_in_names + [partition_name]
    donate = tuple(range(n_params, n_params + n_outs))

    def _body(*args):
        operands = list(args)
        if partition_name is not None:
            operands.append(partition_id_tensor())
        outs = _bass_exec_p.bind(
            *operands,
            out_avals=tuple(out_avals),
            in_names=tuple(all_in_names),
            out_names=tuple(out_names),
            lowering_input_output_aliases=(),
            sim_require_finite=True,
            sim_require_nnan=True,
            nc=nc,
        )
        return tuple(outs)

    try:
        devices = jax.devices("axon")[:n_cores]
    except RuntimeError:
        devices = jax.devices()[:n_cores]
    mesh = Mesh(np.asarray(devices), ("core",))
    sharded = jax.jit(
        shard_map(_body, mesh=mesh,
                  in_specs=(PartitionSpec("core"),) * (n_params + n_outs),
                  out_specs=(PartitionSpec("core"),) * n_outs,
                  check_rep=False),
        donate_argnums=donate,
        keep_unused=True,
    )
    shard0 = NamedSharding(mesh, PartitionSpec("core"))

    def make_zeros():
        return [
            jax.jit(
                lambda shape=av.shape, dt=av.dtype: jnp.zeros(
                    (n_cores * shape[0],) + tuple(shape[1:]), dt),
                out_shardings=shard0,
            )()
            for av in out_avals
        ]

    return sharded, in_names, out_names, out_avals, shard0, make_zeros


def _sort_plan(src, dst):
    """Shared by prep_inputs / assemble_output: the (core, bin) sort."""
    core = src // EC
    iwin = (src % EC) // W
    jwin = dst // W
    key = core * NBIN + iwin * NDW + jwin
    order = np.argsort(key, kind="stable")
    key_s = key[order]
    counts = np.bincount(key, minlength=N_CORES * NBIN)
    mx = int(counts.max())
    if mx > C_BIN:
        raise ValueError(
            f"bin overflow: max bin {mx} > C_BIN {C_BIN}; "
            "raise C_BIN (multiple of 128) and rebuild")
    starts = np.zeros(N_CORES * NBIN, dtype=np.int64)
    np.cumsum(counts[:-1], out=starts[1:])
    r = np.arange(T_FULL, dtype=np.int64) - np.repeat(starts, counts)
    return order, key_s, r


def prep_inputs(o, rbf_env, src_idx, dst_idx):
    """Host-side layout-only prep: dtype packing + sort/bin permutation."""
    o = np.asarray(o, dtype=np.float32)
    rbf = np.asarray(rbf_env, dtype=np.float32)
    src = np.asarray(src_idx).astype(np.int64)
    dst = np.asarray(dst_idx).astype(np.int64)
    assert o.shape == (E_ROWS, 3) and rbf.shape == (E_ROWS, D_OUT)
    assert src.shape == (T_FULL,) and dst.shape == (T_FULL,)

    tblh = np.zeros((E_ROWS, 128), dtype=np.float16)
    tblh[:, :D_OUT] = rbf.astype(np.float16)
    tblh[:, D_OUT:48] = o.view(np.float16)  # raw f32 bytes as 6 fp16 lanes
    o4p = np.zeros((E_ROWS, 64), dtype=np.float32)
    o4p[:, :3] = o

    order, key_s, r = _sort_plan(src, dst)
    _CACHE["plan"] = (order, key_s, r)

    s_loc = (src % W).astype(np.int16)[order]
    d_loc = (dst % W).astype(np.int16)[order]
    S = np.zeros(N_CORES * NBIN * C_BIN, dtype=np.int16)
    D = np.zeros(N_CORES * NBIN * C_BIN, dtype=np.int16)
    flat_pos = key_s * C_BIN + r
    S[flat_pos] = s_loc
    D[flat_pos] = d_loc
    # wrap [bins, C_BIN] -> [bins, 16, C_BIN/16] -> replicate to 128 partitions
    Sw = S.reshape(-1, C_BIN // 16, 16).transpose(0, 2, 1)
    Dw = D.reshape(-1, C_BIN // 16, 16).transpose(0, 2, 1)
    # per (core,bin): [2, 16, 264] (src block then dst block)
    SD = np.stack([Sw, Dw], axis=1)  # [8*128bins, 2, 16, 264]
    SD = SD.reshape(N_CORES, NBIN, 2, 16, C_BIN // 16)
    # -> [core, 16, bins, 2, 264] -> [core, 16, NBIN*528]
    SD = SD.transpose(0, 3, 1, 2, 4).reshape(N_CORES, 16, NBIN * IDXC_BIN)
    idxs_feed = np.tile(SD, (1, 8, 1)).reshape(N_CORES * 128, NBIN * IDXC_BIN)

    concat = {
        "tblh": tblh.reshape(N_CORES * EC, 128),
        "o4p": np.concatenate([o4p] * N_CORES, axis=0),
        "idxs": np.ascontiguousarray(idxs_feed),
    }
    return concat


def assemble_output(out_concat):
    """out_concat: [N_CORES*128, OUT_COLS] fp16 -> [T_FULL, 42] f32."""
    order, key_s, r = _CACHE["plan"]
    res = np.asarray(out_concat)
    # rows = [core, part], cols = [bin, chunk, 42]
    R5 = res.reshape(N_CORES, 128, NBIN, CH_BIN, D_OUT)
    dev = np.ascontiguousarray(R5.transpose(0, 2, 3, 1, 4)).reshape(
        N_CORES * NBIN * C_BIN, D_OUT)
    gathered = dev[key_s * C_BIN + r].astype(np.float32)
    out = np.empty((T_FULL, D_OUT), dtype=np.float32)
    out[order] = gathered
    return out


def kernel(o, rbf_env, src_idx, dst_idx):
    import jax

    if "prog" not in _CACHE:
        _CACHE["prog"] = build_program()
        _CACHE["runner"] = _get_runner(_CACHE["prog"], N_CORES)
    sharded, in_names, out_names, out_avals, shard0, make_zeros = _CACHE["runner"]

    concat = prep_inputs(o, rbf_env, src_idx, dst_idx)
    dev_in = [jax.device_put(concat[name], shard0) for name in in_names]
    outs = sharded(*dev_in, *make_zeros())
    jax.block_until_ready(outs)
    out_concat = np.asarray(outs[out_names.index("out")])
    return assemble_output(out_concat)


# revision 11
# speedup vs baseline: 3.5797x; 1.4050x over previous
"""DimeNet edge_init (DimePredictor) Bass/Trainium2 kernel, v2.

Strategy (8 NeuronCores):
  - Triplets sharded to cores by src-row range (125k rows each), then sorted
    per core by (src-window, dst-window) where windows are 31250 rows (int16
    indexable). 4 src-windows x 32 dst-windows = 128 bins/core, each padded
    to a fixed C_BIN = 4224 slots (zero-index padding; max observed bin
    ~4140 for 4M uniform triplets, asserted at prep time).
  - Per bin, TWO InstDMAGatherAnt ops (single_packet=False, elem < 256B is
    fine in non-transpose mode; HBM row stride must be a multiple of 256B):
      src: 96B rows ([42 x fp16 rbf | 3 x f32 o] from a per-core fp16 table
           slice [125000, 128]) gathered by src%31250,
      dst: 16B rows (f32 [x,y,z,0] from a replicated o table [1M, 64])
           gathered by dst%31250.
    Row i of an instruction lands at partition i%128, chunk i//128.
  - Compute per group of 2 bins (66 chunks of 128 lanes): rbf fp16 -> f32 on
    the ACT engine, angle math in f32 on DVE (dot, norms, rsqrt + one Newton
    step), scaled Legendre recurrence via scalar_tensor_tensor, output fp16.
  - Host: layout-only prep (dtype packing, sort/binning permutation of the
    index data) and inverse permutation + exact fp16->f32 widening of the
    output.
"""
import numpy as np

NUM_SPHERICAL = 7
NUM_RADIAL = 6
D_OUT = NUM_SPHERICAL * NUM_RADIAL  # 42
E_ROWS = 1_000_000
T_FULL = 4_000_000
N_CORES = 8
EC = E_ROWS // N_CORES       # 125_000 src rows per core
W = 31_250                   # window rows (int16-indexable)
NSW = EC // W                # 4 src windows per core
NDW = E_ROWS // W            # 32 dst windows
NBIN = NSW * NDW             # 128 bins per core
C_BIN = 4224                 # slots per bin (33 chunks of 128)
CH_BIN = C_BIN // 128        # 33
BG = 8                       # bins per gather group (8*C_BIN = 33*1024 slots)
N_GROUP = NBIN // BG         # 16 gather groups per core
SB = 2                       # bins per compute sub-tile
N_SUB = BG // SB             # 4 sub-tiles per group
K_G = SB * CH_BIN            # 66 chunk-lanes per compute sub-tile
CH_GRP = BG * CH_BIN         # 264 chunks per gather group
SRCC_G = BG * C_BIN // 16    # 2112 idx cols: group's src slot stream
DSTC_B = C_BIN // 16         # 264 idx cols per bin (dst list)
IDXC_G = SRCC_G + BG * DSTC_B         # 4224 idx cols per group
OUT_COLS = NBIN * CH_BIN * D_OUT      # 177_408 fp16 cols per core

_CACHE = {}


def _dma_gather_raw(gp, out_ap, in_ap, idxs_ap, num_idxs, elem_size, elem_step,
                    queue_num=0, single_packet=True):
    """InstDMAGatherAnt without bass's over-strict elem%256 assert.

    Non-transpose HBM-source gather: elem_size (in elements) arbitrary, the
    table row stride (elem_step * dtype_size) must be a 256B multiple.
    single_packet=True coalesces each engine's descriptors into one SDMA
    packet (64-desc packet cap => num_idxs <= 1024); False makes each
    descriptor its own packet (~140ns/packet on HW - slow for small elems).
    """
    import concourse.mybir as mybir
    from concourse.bass import exact_div

    assert idxs_ap.dtype == mybir.dt.int16
    assert in_ap.dtype == out_ap.dtype
    stride_bytes = elem_step * mybir.dt.size(in_ap.dtype)
    stride_bytes_256 = exact_div(stride_bytes, 256)
    assert 0 < stride_bytes_256 < 256
    _in_ap = gp.lower_ap_dma(in_ap, for_custom_bir_dma=True)
    assert len(_in_ap) == 1
    _idxs_ap = gp.lower_ap(idxs_ap)
    _out_ap = gp.lower_ap(out_ap)
    return gp.add_instruction(
        mybir.InstDMAGatherAnt(
            name=gp.bass.get_next_instruction_name(),
            ins=[*_in_ap, _idxs_ap, gp.lower_val_access(gp.to_reg(num_idxs))],
            outs=[_out_ap],
            transpose=False,
            num_idxs=num_idxs,
            elem_size=elem_size,
            stride_bytes_256=stride_bytes_256,
            gen_mode=0,
            single_packet=single_packet,
            queue_num=queue_num,
            sbuf_tokens_per_rank=0,
            sbuf_free_dim_per_rank=0,
            sbuf_free_dim_pad_per_rank=0,
            sbuf_byte_offset=0,
        )
    )


def build_program(repeat=1, n_cores=N_CORES, c_instr=1024, n_queues=4):
    import concourse.bacc as bacc
    import concourse.tile as tile
    import concourse.mybir as mybir

    f32 = mybir.dt.float32
    f16 = mybir.dt.float16
    i16 = mybir.dt.int16
    mul = mybir.AluOpType.mult
    add = mybir.AluOpType.add

    # per-bin instruction split: c_instr-sized pieces of the C_BIN slot list
    splits = []
    off = 0
    while off < C_BIN:
        sz = min(c_instr, C_BIN - off)
        assert sz % 128 == 0
        splits.append((off, sz))
        off += sz

    nc = bacc.Bacc("TRN2", target_bir_lowering=False, debug=False,
                   num_devices=n_cores, num_swdge_queues=n_queues)
    tblh = nc.dram_tensor("tblh", [EC, 128], f16, kind="ExternalInput").ap()
    o4p = nc.dram_tensor("o4p", [E_ROWS, 64], f32, kind="ExternalInput").ap()
    idxs = nc.dram_tensor("idxs", [128, N_GROUP * IDXC_G], i16,
                          kind="ExternalInput").ap()
    out = nc.dram_tensor("out", [128, OUT_COLS], f16, kind="ExternalOutput").ap()

    # Scaled Legendre recurrence (see baseline): G_l = c*G_{l-1} - b2_l*G_{l-2},
    # emitted output per l uses qscale_l = coef_l * g_l.
    g = [1.0, 1.0]
    for l in range(2, NUM_SPHERICAL):
        g.append(((2 * l - 1) / l) * g[-1])
    b2 = {}
    for l in range(2, NUM_SPHERICAL):
        a_l = (2 * l - 1) / l
        b_l = (l - 1) / l
        b2[l] = b_l * g[l - 2] / (a_l * g[l - 1])
    coef = [float(np.sqrt((2 * l + 1) / (4.0 * np.pi)).astype(np.float32))
            for l in range(NUM_SPHERICAL)]
    qscale = [coef[l] * g[l] for l in range(NUM_SPHERICAL)]

    K = K_G
    rrq = [0]  # round-robin SWDGE queue assignment for gather instructions
    with tile.TileContext(nc) as tc:
        with tc.tile_pool(name="idxp", bufs=3) as idxp, \
             tc.tile_pool(name="ftp", bufs=3) as ftp, \
             tc.tile_pool(name="odp", bufs=3) as odp, \
             tc.tile_pool(name="rbp", bufs=2) as rbp, \
             tc.tile_pool(name="otp", bufs=3) as otp, \
             tc.tile_pool(name="tmp", bufs=2) as tmp:
            for _rep in range(repeat):
                for grp in range(N_GROUP):
                    it = idxp.tile([128, IDXC_G], i16)
                    nc.sync.dma_start(
                        it[:], idxs[:, grp * IDXC_G:(grp + 1) * IDXC_G])

                    ft = ftp.tile([128, CH_GRP * 48], f16)
                    od = odp.tile([128, CH_GRP * 4], f32)
                    ft3 = ft[:].rearrange("p (c e) -> p c e", e=48)
                    od3 = od[:].rearrange("p (c e) -> p c e", e=4)
                    iwin = (grp * BG) // NDW
                    # src side: the group's whole padded slot stream is one
                    # window -> BG*C_BIN/c_instr full-size instructions
                    n_src = (BG * C_BIN) // c_instr
                    assert n_src * c_instr == BG * C_BIN
                    for si in range(n_src):
                        _dma_gather_raw(
                            nc.gpsimd,
                            out_ap=ft3[:, si * (c_instr // 128):
                                       (si + 1) * (c_instr // 128), :],
                            in_ap=tblh[iwin * W:(iwin + 1) * W, 0:48],
                            idxs_ap=it[:, si * (c_instr // 16):
                                       (si + 1) * (c_instr // 16)],
                            num_idxs=c_instr, elem_size=48, elem_step=128,
                            queue_num=rrq[0])
                        rrq[0] = (rrq[0] + 1) % n_queues
                    # dst side: per bin (window-homogeneous), tail split
                    for m in range(BG):
                        b = grp * BG + m
                        jwin = b % NDW
                        c0 = SRCC_G + m * DSTC_B
                        for (soff, ssz) in splits:
                            ch0 = m * CH_BIN + soff // 128
                            _dma_gather_raw(
                                nc.gpsimd,
                                out_ap=od3[:, ch0:ch0 + ssz // 128, :],
                                in_ap=o4p[jwin * W:(jwin + 1) * W, 0:4],
                                idxs_ap=it[:, c0 + soff // 16:
                                           c0 + (soff + ssz) // 16],
                                num_idxs=ssz, elem_size=4, elem_step=64,
                                queue_num=rrq[0])
                            rrq[0] = (rrq[0] + 1) % n_queues

                    ff3g = ft[:].bitcast(f32).rearrange(
                        "p (c e) -> p c e", e=24)
                    for sub in range(N_SUB):
                        s0 = sub * K
                        fv = ft3[:, s0:s0 + K, :]
                        ov = od3[:, s0:s0 + K, :]
                        R1 = ff3g[:, s0:s0 + K, 21:24]
                        R2 = ov[:, :, 0:3]

                        # rbf fp16 -> f32 on the ACT engine
                        rb = rbp.tile([128, K * D_OUT], f32)
                        rb3 = rb[:].rearrange("p (c e) -> p c e", e=D_OUT)
                        nc.scalar.copy(out=rb[:], in_=fv[:, :, 0:D_OUT])

                        m_t = tmp.tile([128, K * 3], f32, tag="m")
                        m3 = m_t[:].rearrange("p (c e) -> p c e", e=3)
                        sc = tmp.tile([128, K * 10], f32, tag="sc")

                        def lane(i):
                            return sc[:, i * K:(i + 1) * K]

                        dot, n1, n2, p_, r_, t_, cc = (lane(i) for i in range(7))
                        gl = [lane(7), lane(8), lane(9)]  # rotating G lanes

                        def lanes_b(ap_flat):
                            return ap_flat.rearrange(
                                "p (c one) -> p c one", one=1).to_broadcast(
                                [128, K, NUM_RADIAL])

                        # dot = R1.R2 ; n1 = |R1|^2 ; n2 = |R2|^2
                        nc.vector.tensor_tensor(out=m3[:], in0=R1, in1=R2, op=mul)
                        nc.vector.tensor_tensor(out=dot, in0=m_t[:, 0::3],
                                                in1=m_t[:, 1::3], op=add)
                        nc.vector.tensor_tensor(out=dot, in0=dot,
                                                in1=m_t[:, 2::3], op=add)
                        nc.vector.tensor_tensor(out=m3[:], in0=R1, in1=R1, op=mul)
                        nc.vector.tensor_tensor(out=n1, in0=m_t[:, 0::3],
                                                in1=m_t[:, 1::3], op=add)
                        nc.vector.tensor_tensor(out=n1, in0=n1,
                                                in1=m_t[:, 2::3], op=add)
                        nc.vector.tensor_tensor(out=m3[:], in0=R2, in1=R2, op=mul)
                        nc.vector.tensor_tensor(out=n2, in0=m_t[:, 0::3],
                                                in1=m_t[:, 1::3], op=add)
                        nc.vector.tensor_tensor(out=n2, in0=n2,
                                                in1=m_t[:, 2::3], op=add)
                        # cc = dot * rsqrt(n1*n2): ACT sqrt + recip + 1 Newton
                        nc.vector.tensor_tensor(out=p_, in0=n1, in1=n2, op=mul)
                        nc.scalar.sqrt(out=r_, in_=p_)
                        nc.vector.reciprocal(out=r_, in_=r_)
                        nc.vector.tensor_tensor(out=t_, in0=r_, in1=r_, op=mul)
                        nc.vector.tensor_tensor(out=t_, in0=t_, in1=p_, op=mul)
                        nc.vector.tensor_scalar(out=t_, in0=t_, scalar1=-0.5,
                                                scalar2=1.5, op0=mul, op1=add)
                        nc.vector.tensor_tensor(out=r_, in0=r_, in1=t_, op=mul)
                        nc.vector.tensor_tensor(out=cc, in0=dot, in1=r_, op=mul)

                        ot = otp.tile([128, K * D_OUT], f16)
                        ot3 = ot[:].rearrange("p (c e) -> p c e", e=D_OUT)

                        def emit(l, G_ap):
                            # out_l = (rbf_l * qscale_l) * G_l
                            nc.vector.scalar_tensor_tensor(
                                out=ot3[:, :, l * NUM_RADIAL:(l + 1) * NUM_RADIAL],
                                in0=rb3[:, :, l * NUM_RADIAL:(l + 1) * NUM_RADIAL],
                                scalar=float(qscale[l]),
                                in1=lanes_b(G_ap),
                                op0=mul, op1=mul)

                        # l = 0: G_0 = 1
                        nc.vector.tensor_scalar(
                            out=ot3[:, :, 0:NUM_RADIAL],
                            in0=rb3[:, :, 0:NUM_RADIAL],
                            scalar1=float(qscale[0]), scalar2=None, op0=mul)
                        # l = 1: G_1 = cc
                        emit(1, cc)
                        # l = 2: G_2 = cc*cc - b2_2
                        nc.vector.tensor_tensor(out=gl[0], in0=cc, in1=cc, op=mul)
                        nc.vector.tensor_scalar(out=gl[0], in0=gl[0],
                                                scalar1=float(-b2[2]),
                                                scalar2=None, op0=add)
                        emit(2, gl[0])
                        # l = 3: G_3 = cc*G_2 - b2_3*G_1 (G_1 = cc)
                        nc.vector.tensor_tensor(out=t_, in0=cc, in1=gl[0], op=mul)
                        nc.vector.scalar_tensor_tensor(
                            out=gl[1], in0=cc, scalar=float(-b2[3]), in1=t_,
                            op0=mul, op1=add)
                        emit(3, gl[1])
                        # l >= 4: G_l = cc*G_{l-1} - b2_l*G_{l-2}
                        for l in range(4, NUM_SPHERICAL):
                            gm1 = gl[(l - 3) % 3]
                            gm2 = gl[(l - 4) % 3]
                            gcur = gl[(l - 2) % 3]
                            nc.vector.tensor_tensor(out=t_, in0=cc, in1=gm1,
                                                    op=mul)
                            nc.vector.scalar_tensor_tensor(
                                out=gcur, in0=gm2, scalar=float(-b2[l]), in1=t_,
                                op0=mul, op1=add)
                            emit(l, gcur)

                        oc0 = (grp * N_SUB + sub) * K * D_OUT
                        nc.sync.dma_start(out[:, oc0:oc0 + K * D_OUT], ot[:])

    nc.compile()
    return nc


def _get_runner(nc, n_cores):
    """Build a jitted SPMD executor for the compiled Bass program."""
    import jax
    import jax.numpy as jnp
    from jax.sharding import Mesh, PartitionSpec, NamedSharding
    from jax.experimental.shard_map import shard_map
    import concourse.mybir as mybir
    from concourse.bass2jax import _bass_exec_p, install_neuronx_cc_hook, partition_id_tensor

    install_neuronx_cc_hook()
    partition_name = nc.partition_id_tensor.name if nc.partition_id_tensor else None
    in_names, out_names, out_avals = [], [], []
    for alloc in nc.m.functions[0].allocations:
        if not isinstance(alloc, mybir.MemoryLocationSet):
            continue
        name = alloc.memorylocations[0].name
        if alloc.kind == "ExternalInput":
            if name != partition_name:
                in_names.append(name)
        elif alloc.kind == "ExternalOutput":
            out_names.append(name)
            out_avals.append(jax.core.ShapedArray(
                tuple(alloc.tensor_shape), mybir.dt.np(alloc.dtype)))
    n_params = len(in_names)
    n_outs = len(out_avals)
    all_in_names = in_names + out_names
    if partition_name is not None:
        all_in_names = all_in_names + [partition_name]
    donate = tuple(range(n_params, n_params + n_outs))

    def _body(*args):
        operands = list(args)
        if partition_name is not None:
            operands.append(partition_id_tensor())
        outs = _bass_exec_p.bind(
            *operands,
            out_avals=tuple(out_avals),
            in_names=tuple(all_in_names),
            out_names=tuple(out_names),
            lowering_input_output_aliases=(),
            sim_require_finite=True,
            sim_require_nnan=True,
            nc=nc,
        )
        return tuple(outs)

    try:
        devices = jax.devices("axon")[:n_cores]
    except RuntimeError:
        devices = jax.devices()[:n_cores]
    mesh = Mesh(np.asarray(devices), ("core",))
    sharded = jax.jit(
        shard_map(_body, mesh=mesh,
                  in_specs=(PartitionSpec("core"),) * (n_params + n_outs),
                  out_specs=(PartitionSpec("core"),) * n_outs,
                  check_rep=False),
        donate_argnums=donate,
        keep_unused=True,
    )
    shard0 = NamedSharding(mesh, PartitionSpec("core"))

    def make_zeros():
        return [
            jax.jit(
                lambda shape=av.shape, dt=av.dtype: jnp.zeros(
                    (n_cores * shape[0],) + tuple(shape[1:]), dt),
                out_shardings=shard0,
            )()
            for av in out_avals
        ]

    return sharded, in_names, out_names, out_avals, shard0, make_zeros


def _sort_plan(src, dst):
    """Shared by prep_inputs / assemble_output: the (core, bin) sort."""
    core = src // EC
    iwin = (src % EC) // W
    jwin = dst // W
    key = core * NBIN + iwin * NDW + jwin
    order = np.argsort(key, kind="stable")
    key_s = key[order]
    counts = np.bincount(key, minlength=N_CORES * NBIN)
    mx = int(counts.max())
    if mx > C_BIN:
        raise ValueError(
            f"bin overflow: max bin {mx} > C_BIN {C_BIN}; "
            "raise C_BIN (multiple of 128) and rebuild")
    starts = np.zeros(N_CORES * NBIN, dtype=np.int64)
    np.cumsum(counts[:-1], out=starts[1:])
    r = np.arange(T_FULL, dtype=np.int64) - np.repeat(starts, counts)
    return order, key_s, r


def prep_inputs(o, rbf_env, src_idx, dst_idx):
    """Host-side layout-only prep: dtype packing + sort/bin permutation."""
    o = np.asarray(o, dtype=np.float32)
    rbf = np.asarray(rbf_env, dtype=np.float32)
    src = np.asarray(src_idx).astype(np.int64)
    dst = np.asarray(dst_idx).astype(np.int64)
    assert o.shape == (E_ROWS, 3) and rbf.shape == (E_ROWS, D_OUT)
    assert src.shape == (T_FULL,) and dst.shape == (T_FULL,)

    tblh = np.zeros((E_ROWS, 128), dtype=np.float16)
    tblh[:, :D_OUT] = rbf.astype(np.float16)
    tblh[:, D_OUT:48] = o.view(np.float16)  # raw f32 bytes as 6 fp16 lanes
    o4p = np.zeros((E_ROWS, 64), dtype=np.float32)
    o4p[:, :3] = o

    order, key_s, r = _sort_plan(src, dst)
    _CACHE["plan"] = (order, key_s, r)

    s_loc = (src % W).astype(np.int16)[order]
    d_loc = (dst % W).astype(np.int16)[order]
    S = np.zeros(N_CORES * NBIN * C_BIN, dtype=np.int16)
    D = np.zeros(N_CORES * NBIN * C_BIN, dtype=np.int16)
    flat_pos = key_s * C_BIN + r
    S[flat_pos] = s_loc
    D[flat_pos] = d_loc
    # wrap [bins, C_BIN] -> [bins, 16, C_BIN/16] -> replicate to 128 partitions
    # src: one wrapped stream per gather group (BG bins' padded lists)
    Sg = S.reshape(N_CORES, N_GROUP, BG * C_BIN // 16, 16).transpose(0, 1, 3, 2)
    # dst: per-bin wrapped lists, bin-major within the group
    Dg = D.reshape(N_CORES, N_GROUP, BG, C_BIN // 16, 16).transpose(
        0, 1, 4, 2, 3).reshape(N_CORES, N_GROUP, 16, BG * DSTC_B)
    SD = np.concatenate([Sg, Dg], axis=3)      # [core, grp, 16, IDXC_G]
    SD = SD.transpose(0, 2, 1, 3).reshape(N_CORES, 16, N_GROUP * IDXC_G)
    idxs_feed = np.tile(SD, (1, 8, 1)).reshape(N_CORES * 128, N_GROUP * IDXC_G)

    concat = {
        "tblh": tblh.reshape(N_CORES * EC, 128),
        "o4p": np.concatenate([o4p] * N_CORES, axis=0),
        "idxs": np.ascontiguousarray(idxs_feed),
    }
    return concat


def assemble_output(out_concat):
    """out_concat: [N_CORES*128, OUT_COLS] fp16 -> [T_FULL, 42] f32."""
    order, key_s, r = _CACHE["plan"]
    res = np.asarray(out_concat)
    # rows = [core, part], cols = [bin, chunk, 42]
    R5 = res.reshape(N_CORES, 128, NBIN, CH_BIN, D_OUT)
    dev = np.ascontiguousarray(R5.transpose(0, 2, 3, 1, 4)).reshape(
        N_CORES * NBIN * C_BIN, D_OUT)
    gathered = dev[key_s * C_BIN + r].astype(np.float32)
    out = np.empty((T_FULL, D_OUT), dtype=np.float32)
    out[order] = gathered
    return out


def kernel(o, rbf_env, src_idx, dst_idx):
    import jax

    if "prog" not in _CACHE:
        _CACHE["prog"] = build_program()
        _CACHE["runner"] = _get_runner(_CACHE["prog"], N_CORES)
    sharded, in_names, out_names, out_avals, shard0, make_zeros = _CACHE["runner"]

    concat = prep_inputs(o, rbf_env, src_idx, dst_idx)
    dev_in = [jax.device_put(concat[name], shard0) for name in in_names]
    outs = sharded(*dev_in, *make_zeros())
    jax.block_until_ready(outs)
    out_concat = np.asarray(outs[out_names.index("out")])
    return assemble_output(out_concat)


# revision 12
# speedup vs baseline: 3.5818x; 1.0006x over previous
"""DimeNet edge_init (DimePredictor) Bass/Trainium2 kernel, v2.

Strategy (8 NeuronCores):
  - Triplets sharded to cores by src-row range (125k rows each), then sorted
    per core by (src-window, dst-window) where windows are 31250 rows (int16
    indexable). 4 src-windows x 32 dst-windows = 128 bins/core, each padded
    to a fixed C_BIN = 4224 slots (zero-index padding; max observed bin
    ~4140 for 4M uniform triplets, asserted at prep time).
  - Gathers use raw InstDMAGatherAnt (elem < 256B is fine in non-transpose
    mode; HBM row stride must be a 256B multiple; single_packet=True caps an
    instruction at 1024 indices), round-robined over all 4 SWDGE queues so
    the Q7 descriptor-gen pairs run in parallel:
      src: 96B rows ([42 x fp16 rbf | 3 x f32 o] from a per-core fp16 table
           slice [125000, 128]) gathered by src%31250 — issued as 33
           full-1024 instructions per 8-bin group (8*4224 = 33*1024, no
           tail instructions),
      dst: 16B rows (f32 [x,y,z,0] from a replicated o table [1M, 64])
           gathered by dst%31250 — per bin, 4x1024 + 1x128.
    Row i of an instruction lands at partition i%128, chunk i//128.
  - Compute per sub-tile of 2 bins (66 chunks of 128 lanes): rbf fp16 -> f32
    on the ACT engine, angle math in f32 on DVE (dot, norms, rsqrt + one
    Newton step), scaled Legendre recurrence via scalar_tensor_tensor,
    output fp16.
  - Host: layout-only prep (dtype packing, sort/binning permutation of the
    index data) and inverse permutation + exact fp16->f32 widening of the
    output.
"""
import numpy as np

NUM_SPHERICAL = 7
NUM_RADIAL = 6
D_OUT = NUM_SPHERICAL * NUM_RADIAL  # 42
E_ROWS = 1_000_000
T_FULL = 4_000_000
N_CORES = 8
EC = E_ROWS // N_CORES       # 125_000 src rows per core
W = 31_250                   # window rows (int16-indexable)
NSW = EC // W                # 4 src windows per core
NDW = E_ROWS // W            # 32 dst windows
NBIN = NSW * NDW             # 128 bins per core
C_BIN = 4224                 # slots per bin (33 chunks of 128)
CH_BIN = C_BIN // 128        # 33
BG = 8                       # bins per gather group (8*C_BIN = 33*1024 slots)
N_GROUP = NBIN // BG         # 16 gather groups per core
SB = 2                       # bins per compute sub-tile
N_SUB = BG // SB             # 4 sub-tiles per group
K_G = SB * CH_BIN            # 66 chunk-lanes per compute sub-tile
CH_GRP = BG * CH_BIN         # 264 chunks per gather group
SRCC_G = BG * C_BIN // 16    # 2112 idx cols: group's src slot stream
DSTC_B = C_BIN // 16         # 264 idx cols per bin (dst list)
IDXC_G = SRCC_G + BG * DSTC_B         # 4224 idx cols per group
OUT_COLS = NBIN * CH_BIN * D_OUT      # 177_408 fp16 cols per core

_CACHE = {}


def _dma_gather_raw(gp, out_ap, in_ap, idxs_ap, num_idxs, elem_size, elem_step,
                    queue_num=0, single_packet=True):
    """InstDMAGatherAnt without bass's over-strict elem%256 assert.

    Non-transpose HBM-source gather: elem_size (in elements) arbitrary, the
    table row stride (elem_step * dtype_size) must be a 256B multiple.
    single_packet=True coalesces each engine's descriptors into one SDMA
    packet (64-desc packet cap => num_idxs <= 1024); False makes each
    descriptor its own packet (~140ns/packet on HW - slow for small elems).
    """
    import concourse.mybir as mybir
    from concourse.bass import exact_div

    assert idxs_ap.dtype == mybir.dt.int16
    assert in_ap.dtype == out_ap.dtype
    stride_bytes = elem_step * mybir.dt.size(in_ap.dtype)
    stride_bytes_256 = exact_div(stride_bytes, 256)
    assert 0 < stride_bytes_256 < 256
    _in_ap = gp.lower_ap_dma(in_ap, for_custom_bir_dma=True)
    assert len(_in_ap) == 1
    _idxs_ap = gp.lower_ap(idxs_ap)
    _out_ap = gp.lower_ap(out_ap)
    return gp.add_instruction(
        mybir.InstDMAGatherAnt(
            name=gp.bass.get_next_instruction_name(),
            ins=[*_in_ap, _idxs_ap, gp.lower_val_access(gp.to_reg(num_idxs))],
            outs=[_out_ap],
            transpose=False,
            num_idxs=num_idxs,
            elem_size=elem_size,
            stride_bytes_256=stride_bytes_256,
            gen_mode=0,
            single_packet=single_packet,
            queue_num=queue_num,
            sbuf_tokens_per_rank=0,
            sbuf_free_dim_per_rank=0,
            sbuf_free_dim_pad_per_rank=0,
            sbuf_byte_offset=0,
        )
    )


def build_program(repeat=1, n_cores=N_CORES, c_instr=1024, n_queues=4):
    import concourse.bacc as bacc
    import concourse.tile as tile
    import concourse.mybir as mybir

    f32 = mybir.dt.float32
    f16 = mybir.dt.float16
    i16 = mybir.dt.int16
    mul = mybir.AluOpType.mult
    add = mybir.AluOpType.add

    # per-bin instruction split: c_instr-sized pieces of the C_BIN slot list
    splits = []
    off = 0
    while off < C_BIN:
        sz = min(c_instr, C_BIN - off)
        assert sz % 128 == 0
        splits.append((off, sz))
        off += sz

    nc = bacc.Bacc("TRN2", target_bir_lowering=False, debug=False,
                   num_devices=n_cores, num_swdge_queues=n_queues)
    tblh = nc.dram_tensor("tblh", [EC, 128], f16, kind="ExternalInput").ap()
    o4p = nc.dram_tensor("o4p", [E_ROWS, 64], f32, kind="ExternalInput").ap()
    idxs = nc.dram_tensor("idxs", [128, N_GROUP * IDXC_G], i16,
                          kind="ExternalInput").ap()
    out = nc.dram_tensor("out", [128, OUT_COLS], f16, kind="ExternalOutput").ap()

    # Scaled Legendre recurrence (see baseline): G_l = c*G_{l-1} - b2_l*G_{l-2},
    # emitted output per l uses qscale_l = coef_l * g_l.
    g = [1.0, 1.0]
    for l in range(2, NUM_SPHERICAL):
        g.append(((2 * l - 1) / l) * g[-1])
    b2 = {}
    for l in range(2, NUM_SPHERICAL):
        a_l = (2 * l - 1) / l
        b_l = (l - 1) / l
        b2[l] = b_l * g[l - 2] / (a_l * g[l - 1])
    coef = [float(np.sqrt((2 * l + 1) / (4.0 * np.pi)).astype(np.float32))
            for l in range(NUM_SPHERICAL)]
    qscale = [coef[l] * g[l] for l in range(NUM_SPHERICAL)]

    K = K_G
    rrq = [0]  # round-robin SWDGE queue assignment for gather instructions
    with tile.TileContext(nc) as tc:
        with tc.tile_pool(name="idxp", bufs=3) as idxp, \
             tc.tile_pool(name="ftp", bufs=3) as ftp, \
             tc.tile_pool(name="odp", bufs=3) as odp, \
             tc.tile_pool(name="rbp", bufs=2) as rbp, \
             tc.tile_pool(name="otp", bufs=3) as otp, \
             tc.tile_pool(name="tmp", bufs=2) as tmp:
            for _rep in range(repeat):
                for grp in range(N_GROUP):
                    it = idxp.tile([128, IDXC_G], i16)
                    nc.sync.dma_start(
                        it[:], idxs[:, grp * IDXC_G:(grp + 1) * IDXC_G])

                    ft = ftp.tile([128, CH_GRP * 48], f16)
                    od = odp.tile([128, CH_GRP * 4], f32)
                    ft3 = ft[:].rearrange("p (c e) -> p c e", e=48)
                    od3 = od[:].rearrange("p (c e) -> p c e", e=4)
                    iwin = (grp * BG) // NDW
                    # src side: the group's whole padded slot stream is one
                    # window -> BG*C_BIN/c_instr full-size instructions
                    n_src = (BG * C_BIN) // c_instr
                    assert n_src * c_instr == BG * C_BIN
                    for si in range(n_src):
                        _dma_gather_raw(
                            nc.gpsimd,
                            out_ap=ft3[:, si * (c_instr // 128):
                                       (si + 1) * (c_instr // 128), :],
                            in_ap=tblh[iwin * W:(iwin + 1) * W, 0:48],
                            idxs_ap=it[:, si * (c_instr // 16):
                                       (si + 1) * (c_instr // 16)],
                            num_idxs=c_instr, elem_size=48, elem_step=128,
                            queue_num=rrq[0])
                        rrq[0] = (rrq[0] + 1) % n_queues
                    # dst side: per bin (window-homogeneous), tail split
                    for m in range(BG):
                        b = grp * BG + m
                        jwin = b % NDW
                        c0 = SRCC_G + m * DSTC_B
                        for (soff, ssz) in splits:
                            ch0 = m * CH_BIN + soff // 128
                            _dma_gather_raw(
                                nc.gpsimd,
                                out_ap=od3[:, ch0:ch0 + ssz // 128, :],
                                in_ap=o4p[jwin * W:(jwin + 1) * W, 0:4],
                                idxs_ap=it[:, c0 + soff // 16:
                                           c0 + (soff + ssz) // 16],
                                num_idxs=ssz, elem_size=4, elem_step=64,
                                queue_num=rrq[0])
                            rrq[0] = (rrq[0] + 1) % n_queues

                    ff3g = ft[:].bitcast(f32).rearrange(
                        "p (c e) -> p c e", e=24)
                    for sub in range(N_SUB):
                        s0 = sub * K
                        fv = ft3[:, s0:s0 + K, :]
                        ov = od3[:, s0:s0 + K, :]
                        R1 = ff3g[:, s0:s0 + K, 21:24]
                        R2 = ov[:, :, 0:3]

                        # rbf fp16 -> f32 on the ACT engine
                        rb = rbp.tile([128, K * D_OUT], f32)
                        rb3 = rb[:].rearrange("p (c e) -> p c e", e=D_OUT)
                        nc.scalar.copy(out=rb[:], in_=fv[:, :, 0:D_OUT])

                        m_t = tmp.tile([128, K * 3], f32, tag="m")
                        m3 = m_t[:].rearrange("p (c e) -> p c e", e=3)
                        sc = tmp.tile([128, K * 10], f32, tag="sc")

                        def lane(i):
                            return sc[:, i * K:(i + 1) * K]

                        dot, n1, n2, p_, r_, t_, cc = (lane(i) for i in range(7))
                        gl = [lane(7), lane(8), lane(9)]  # rotating G lanes

                        def lanes_b(ap_flat):
                            return ap_flat.rearrange(
                                "p (c one) -> p c one", one=1).to_broadcast(
                                [128, K, NUM_RADIAL])

                        # dot = R1.R2 ; n1 = |R1|^2 ; n2 = |R2|^2
                        nc.vector.tensor_tensor(out=m3[:], in0=R1, in1=R2, op=mul)
                        nc.vector.tensor_tensor(out=dot, in0=m_t[:, 0::3],
                                                in1=m_t[:, 1::3], op=add)
                        nc.vector.tensor_tensor(out=dot, in0=dot,
                                                in1=m_t[:, 2::3], op=add)
                        nc.vector.tensor_tensor(out=m3[:], in0=R1, in1=R1, op=mul)
                        nc.vector.tensor_tensor(out=n1, in0=m_t[:, 0::3],
                                                in1=m_t[:, 1::3], op=add)
                        nc.vector.tensor_tensor(out=n1, in0=n1,
                                                in1=m_t[:, 2::3], op=add)
                        nc.vector.tensor_tensor(out=m3[:], in0=R2, in1=R2, op=mul)
                        nc.vector.tensor_tensor(out=n2, in0=m_t[:, 0::3],
                                                in1=m_t[:, 1::3], op=add)
                        nc.vector.tensor_tensor(out=n2, in0=n2,
                                                in1=m_t[:, 2::3], op=add)
                        # cc = dot * rsqrt(n1*n2): ACT sqrt + recip + 1 Newton
                        nc.vector.tensor_tensor(out=p_, in0=n1, in1=n2, op=mul)
                        nc.scalar.sqrt(out=r_, in_=p_)
                        nc.vector.reciprocal(out=r_, in_=r_)
                        nc.vector.tensor_tensor(out=t_, in0=r_, in1=r_, op=mul)
                        nc.vector.tensor_tensor(out=t_, in0=t_, in1=p_, op=mul)
                        nc.vector.tensor_scalar(out=t_, in0=t_, scalar1=-0.5,
                                                scalar2=1.5, op0=mul, op1=add)
                        nc.vector.tensor_tensor(out=r_, in0=r_, in1=t_, op=mul)
                        nc.vector.tensor_tensor(out=cc, in0=dot, in1=r_, op=mul)

                        ot = otp.tile([128, K * D_OUT], f16)
                        ot3 = ot[:].rearrange("p (c e) -> p c e", e=D_OUT)

                        def emit(l, G_ap):
                            # out_l = (rbf_l * qscale_l) * G_l
                            nc.vector.scalar_tensor_tensor(
                                out=ot3[:, :, l * NUM_RADIAL:(l + 1) * NUM_RADIAL],
                                in0=rb3[:, :, l * NUM_RADIAL:(l + 1) * NUM_RADIAL],
                                scalar=float(qscale[l]),
                                in1=lanes_b(G_ap),
                                op0=mul, op1=mul)

                        # l = 0: G_0 = 1
                        nc.vector.tensor_scalar(
                            out=ot3[:, :, 0:NUM_RADIAL],
                            in0=rb3[:, :, 0:NUM_RADIAL],
                            scalar1=float(qscale[0]), scalar2=None, op0=mul)
                        # l = 1: G_1 = cc
                        emit(1, cc)
                        # l = 2: G_2 = cc*cc - b2_2
                        nc.vector.tensor_tensor(out=gl[0], in0=cc, in1=cc, op=mul)
                        nc.vector.tensor_scalar(out=gl[0], in0=gl[0],
                                                scalar1=float(-b2[2]),
                                                scalar2=None, op0=add)
                        emit(2, gl[0])
                        # l = 3: G_3 = cc*G_2 - b2_3*G_1 (G_1 = cc)
                        nc.vector.tensor_tensor(out=t_, in0=cc, in1=gl[0], op=mul)
                        nc.vector.scalar_tensor_tensor(
                            out=gl[1], in0=cc, scalar=float(-b2[3]), in1=t_,
                            op0=mul, op1=add)
                        emit(3, gl[1])
                        # l >= 4: G_l = cc*G_{l-1} - b2_l*G_{l-2}
                        for l in range(4, NUM_SPHERICAL):
                            gm1 = gl[(l - 3) % 3]
                            gm2 = gl[(l - 4) % 3]
                            gcur = gl[(l - 2) % 3]
                            nc.vector.tensor_tensor(out=t_, in0=cc, in1=gm1,
                                                    op=mul)
                            nc.vector.scalar_tensor_tensor(
                                out=gcur, in0=gm2, scalar=float(-b2[l]), in1=t_,
                                op0=mul, op1=add)
                            emit(l, gcur)

                        oc0 = (grp * N_SUB + sub) * K * D_OUT
                        nc.sync.dma_start(out[:, oc0:oc0 + K * D_OUT], ot[:])

    nc.compile()
    return nc


def _get_runner(nc, n_cores):
    """Build a jitted SPMD executor for the compiled Bass program."""
    import jax
    import jax.numpy as jnp
    from jax.sharding import Mesh, PartitionSpec, NamedSharding
    from jax.experimental.shard_map import shard_map
    import concourse.mybir as mybir
    from concourse.bass2jax import _bass_exec_p, install_neuronx_cc_hook, partition_id_tensor

    install_neuronx_cc_hook()
    partition_name = nc.partition_id_tensor.name if nc.partition_id_tensor else None
    in_names, out_names, out_avals = [], [], []
    for alloc in nc.m.functions[0].allocations:
        if not isinstance(alloc, mybir.MemoryLocationSet):
            continue
        name = alloc.memorylocations[0].name
        if alloc.kind == "ExternalInput":
            if name != partition_name:
                in_names.append(name)
        elif alloc.kind == "ExternalOutput":
            out_names.append(name)
            out_avals.append(jax.core.ShapedArray(
                tuple(alloc.tensor_shape), mybir.dt.np(alloc.dtype)))
    n_params = len(in_names)
    n_outs = len(out_avals)
    all_in_names = in_names + out_names
    if partition_name is not None:
        all_in_names = all_in_names + [partition_name]
    donate = tuple(range(n_params, n_params + n_outs))

    def _body(*args):
        operands = list(args)
        if partition_name is not None:
            operands.append(partition_id_tensor())
        outs = _bass_exec_p.bind(
            *operands,
            out_avals=tuple(out_avals),
            in_names=tuple(all_in_names),
            out_names=tuple(out_names),
            lowering_input_output_aliases=(),
            sim_require_finite=True,
            sim_require_nnan=True,
            nc=nc,
        )
        return tuple(outs)

    try:
        devices = jax.devices("axon")[:n_cores]
    except RuntimeError:
        devices = jax.devices()[:n_cores]
    mesh = Mesh(np.asarray(devices), ("core",))
    sharded = jax.jit(
        shard_map(_body, mesh=mesh,
                  in_specs=(PartitionSpec("core"),) * (n_params + n_outs),
                  out_specs=(PartitionSpec("core"),) * n_outs,
                  check_rep=False),
        donate_argnums=donate,
        keep_unused=True,
    )
    shard0 = NamedSharding(mesh, PartitionSpec("core"))

    def make_zeros():
        return [
            jax.jit(
                lambda shape=av.shape, dt=av.dtype: jnp.zeros(
                    (n_cores * shape[0],) + tuple(shape[1:]), dt),
                out_shardings=shard0,
            )()
            for av in out_avals
        ]

    return sharded, in_names, out_names, out_avals, shard0, make_zeros


def _sort_plan(src, dst):
    """Shared by prep_inputs / assemble_output: the (core, bin) sort."""
    core = src // EC
    iwin = (src % EC) // W
    jwin = dst // W
    key = core * NBIN + iwin * NDW + jwin
    order = np.argsort(key, kind="stable")
    key_s = key[order]
    counts = np.bincount(key, minlength=N_CORES * NBIN)
    mx = int(counts.max())
    if mx > C_BIN:
        raise ValueError(
            f"bin overflow: max bin {mx} > C_BIN {C_BIN}; "
            "raise C_BIN (multiple of 128) and rebuild")
    starts = np.zeros(N_CORES * NBIN, dtype=np.int64)
    np.cumsum(counts[:-1], out=starts[1:])
    r = np.arange(T_FULL, dtype=np.int64) - np.repeat(starts, counts)
    return order, key_s, r


def prep_inputs(o, rbf_env, src_idx, dst_idx):
    """Host-side layout-only prep: dtype packing + sort/bin permutation."""
    o = np.asarray(o, dtype=np.float32)
    rbf = np.asarray(rbf_env, dtype=np.float32)
    src = np.asarray(src_idx).astype(np.int64)
    dst = np.asarray(dst_idx).astype(np.int64)
    assert o.shape == (E_ROWS, 3) and rbf.shape == (E_ROWS, D_OUT)
    assert src.shape == (T_FULL,) and dst.shape == (T_FULL,)

    tblh = np.zeros((E_ROWS, 128), dtype=np.float16)
    tblh[:, :D_OUT] = rbf.astype(np.float16)
    tblh[:, D_OUT:48] = o.view(np.float16)  # raw f32 bytes as 6 fp16 lanes
    o4p = np.zeros((E_ROWS, 64), dtype=np.float32)
    o4p[:, :3] = o

    order, key_s, r = _sort_plan(src, dst)
    _CACHE["plan"] = (order, key_s, r)

    s_loc = (src % W).astype(np.int16)[order]
    d_loc = (dst % W).astype(np.int16)[order]
    S = np.zeros(N_CORES * NBIN * C_BIN, dtype=np.int16)
    D = np.zeros(N_CORES * NBIN * C_BIN, dtype=np.int16)
    flat_pos = key_s * C_BIN + r
    S[flat_pos] = s_loc
    D[flat_pos] = d_loc
    # wrap [bins, C_BIN] -> [bins, 16, C_BIN/16] -> replicate to 128 partitions
    # src: one wrapped stream per gather group (BG bins' padded lists)
    Sg = S.reshape(N_CORES, N_GROUP, BG * C_BIN // 16, 16).transpose(0, 1, 3, 2)
    # dst: per-bin wrapped lists, bin-major within the group
    Dg = D.reshape(N_CORES, N_GROUP, BG, C_BIN // 16, 16).transpose(
        0, 1, 4, 2, 3).reshape(N_CORES, N_GROUP, 16, BG * DSTC_B)
    SD = np.concatenate([Sg, Dg], axis=3)      # [core, grp, 16, IDXC_G]
    SD = SD.transpose(0, 2, 1, 3).reshape(N_CORES, 16, N_GROUP * IDXC_G)
    idxs_feed = np.tile(SD, (1, 8, 1)).reshape(N_CORES * 128, N_GROUP * IDXC_G)

    concat = {
        "tblh": tblh.reshape(N_CORES * EC, 128),
        "o4p": np.concatenate([o4p] * N_CORES, axis=0),
        "idxs": np.ascontiguousarray(idxs_feed),
    }
    return concat


def assemble_output(out_concat):
    """out_concat: [N_CORES*128, OUT_COLS] fp16 -> [T_FULL, 42] f32."""
    order, key_s, r = _CACHE["plan"]
    res = np.asarray(out_concat)
    # rows = [core, part], cols = [bin, chunk, 42]
    R5 = res.reshape(N_CORES, 128, NBIN, CH_BIN, D_OUT)
    dev = np.ascontiguousarray(R5.transpose(0, 2, 3, 1, 4)).reshape(
        N_CORES * NBIN * C_BIN, D_OUT)
    gathered = dev[key_s * C_BIN + r].astype(np.float32)
    out = np.empty((T_FULL, D_OUT), dtype=np.float32)
    out[order] = gathered
    return out


def kernel(o, rbf_env, src_idx, dst_idx):
    import jax

    if "prog" not in _CACHE:
        _CACHE["prog"] = build_program()
        _CACHE["runner"] = _get_runner(_CACHE["prog"], N_CORES)
    sharded, in_names, out_names, out_avals, shard0, make_zeros = _CACHE["runner"]

    concat = prep_inputs(o, rbf_env, src_idx, dst_idx)
    dev_in = [jax.device_put(concat[name], shard0) for name in in_names]
    outs = sharded(*dev_in, *make_zeros())
    jax.block_until_ready(outs)
    out_concat = np.asarray(outs[out_names.index("out")])
    return assemble_output(out_concat)
